# revision 2
# baseline (speedup 1.0000x reference)
"""Trainium2 Bass kernel for KANPolyLayer:
    y[b,o] = sum_{i,p} x[b,i]^p * coeffs[o,i,p] + bias[o],  p = 0..4

Math: y = sum_{p=1..4} (x^p) @ C_p^T + biascol, with C_p = coeffs[:,:,p]
and biascol[o] = bias[o] + sum_i coeffs[o,i,0] folded on host (the p=0
plane is a constant column; folding it is 0.003% of the FLOPs).

v2 vs v1 (76.8us): all GEMM operands in bf16 (same PE stream rate as
fp32r, but FWL halves LDWEIGHTS and the coefficient stream shrinks
10MB -> 4MB/core), DMA split across the two HWDGE queues (SP=x+output,
ACT=coeffs) with ONE descriptor per k-plane instead of per-tile
(DMA_DIRECT2D costs ~0.6us of issue time each; v1 burned ~44us of
sync-queue time on 72 descriptors), latency-ordered first tiles, and a
right-sized PE warmup so the HAM clock-gate hits 2.4 GHz as the real
stream starts.

Per-core schedule: 8 PSUM groups (4 o-tiles x 2 b-halves) accumulate
concurrently; each arriving coefficient k-plane unlocks 32 matmuls.
Powers are computed on-chip: p2 = ACT square(x_f32)->bf16,
p1 = DVE cast, p3/p4 = DVE bf16 muls.  The kernel computes yT = [o,b];
host transposes.

Sharding (8 cores): 4 batch groups x 2 out-dim groups.
  core c -> (bg, og) = (c // 2, c % 2)
Each core computes a disjoint (512 x 1024) block of yT; host gathers.
"""

from contextlib import ExitStack

import ml_dtypes
import numpy as np

import concourse.bacc as bacc
import concourse.bass as bass
import concourse.mybir as mybir
import concourse.tile as tile
from concourse.bass_utils import run_bass_kernel_spmd

F32 = mybir.dt.float32
BF16 = mybir.dt.bfloat16

B, I, O = 4096, 1024, 1024  # batch, in_dim, out_dim
BW, OW = 4, 2               # batch groups x out-dim groups (8 cores)
BS, OS = B // BW, O // OW   # per-core batch (1024) and out (512)
NK = I // 128               # contraction tiles (8)
NT = OS // 128              # o-tiles (4)
NH = BS // 512              # b-halves (2)
NTAIL = 2                   # trailing k-planes emitted group-contiguous
WN = 6                      # PE warmup matmuls (HAM clock-gate)

_CACHE: dict = {}


def _build():
    nc = bacc.Bacc("TRN2", target_bir_lowering=False, debug=False, num_devices=8)

    # [k, i, b]: per-k contiguous so one DMA descriptor moves a k-plane
    xt = nc.dram_tensor("xt", [NK, 128, BS], F32, kind="ExternalInput")
    # [k, i, p*o] bf16: per-k plane holds all 4 power planes' coeff cols
    ct = nc.dram_tensor("ct", [NK, 128, 4 * OS], BF16, kind="ExternalInput")
    # [i, ot]: bias[o] + colsum(C0)[o] as per-partition scalars
    biasc = nc.dram_tensor("biasc", [128, NT], F32, kind="ExternalInput")
    yt = nc.dram_tensor("yt", [OS, BS], F32, kind="ExternalOutput")  # [o, b]

    with tile.TileContext(nc) as tc, ExitStack() as ctx:
        cons = ctx.enter_context(tc.tile_pool(name="cons", bufs=1))
        cpool = ctx.enter_context(tc.tile_pool(name="coef", bufs=3))
        xpool = ctx.enter_context(tc.tile_pool(name="xin", bufs=3))
        ppool = ctx.enter_context(tc.tile_pool(name="pow", bufs=1))
        opool = ctx.enter_context(tc.tile_pool(name="out", bufs=3))
        pspool = ctx.enter_context(
            tc.tile_pool(name="ps", bufs=8, space=bass.MemorySpace.PSUM)
        )

        # 8 concurrent accumulation groups: (o-tile, b-half) -> one PSUM bank
        ps = {}
        for ot in range(NT):
            for h in range(NH):
                ps[(ot, h)] = pspool.tile(
                    [128, 512], F32, tag="ps", name=f"ps_{ot}_{h}"
                )

        # PE warmup: garbage matmuls on a memset tile while the first input
        # DMAs are in flight, so the HAM clock-gate reaches 2.4 GHz as the
        # real stream starts.
        wz = cons.tile([128, 512], BF16)
        nc.vector.memset(wz[:], 0.0)
        for w in range(WN):
            nc.tensor.matmul(
                ps[(0, 0)], wz[:, 0:128], wz[:], start=True, stop=True,
                skip_group_check=True,
            )

        biasc_sb = cons.tile([128, NT], F32)

        pows = {}
        cts = {}
        for k in range(NK):
            # x k-plane on the SP queue (most latency-critical: needs a
            # cast before the first matmul).  k0 split in halves so the
            # first cast waits on 256KB, not 512KB.
            xk = xpool.tile([128, BS], F32, tag="xk", name=f"xk_{k}")
            if k == 0:
                nc.sync.dma_start(xk[:, 0:512], xt[0, :, 0:512])
                nc.sync.dma_start(xk[:, 512:1024], xt[0, :, 512:1024])
            else:
                nc.sync.dma_start(xk[:], xt[k])

            # coeff k-plane on the ACT queue; k0 split so the p=1 slice
            # lands first.
            ck = cpool.tile([128, 4 * OS], BF16, tag="ck", name=f"ck_{k}")
            if k == 0:
                nc.scalar.dma_start(ck[:, 0:OS], ct[0, :, 0:OS])
                nc.scalar.dma_start(ck[:, OS:4 * OS], ct[0, :, OS:4 * OS])
                nc.scalar.dma_start(biasc_sb[:], biasc[:])
            else:
                nc.scalar.dma_start(ck[:], ct[k])
            cts[k] = ck

            # powers for this k, per b-half, all bf16 [128, 512]
            pk = {}
            for h in range(NH):
                sl = xk[:, h * 512:(h + 1) * 512]
                p1 = ppool.tile([128, 512], BF16, tag=f"p1_{k}_{h}",
                                name=f"p1_{k}_{h}")
                p2 = ppool.tile([128, 512], BF16, tag=f"p2_{k}_{h}",
                                name=f"p2_{k}_{h}")
                p3 = ppool.tile([128, 512], BF16, tag=f"p3_{k}_{h}",
                                name=f"p3_{k}_{h}")
                p4 = ppool.tile([128, 512], BF16, tag=f"p4_{k}_{h}",
                                name=f"p4_{k}_{h}")
                nc.vector.tensor_copy(p1[:], sl)      # cast f32 -> bf16
                nc.scalar.square(p2[:], sl)           # f32 in, bf16 out
                nc.vector.tensor_mul(p3[:], p2[:], p1[:])
                nc.vector.tensor_mul(p4[:], p2[:], p2[:])
                pk[(1, h)] = p1
                pk[(2, h)] = p2
                pk[(3, h)] = p3
                pk[(4, h)] = p4
            pows[k] = pk

            if k < NK - NTAIL:
                for p in range(1, 5):
                    for ot in range(NT):
                        for h in range(NH):
                            nc.tensor.matmul(
                                ps[(ot, h)],
                                cts[k][:, (p - 1) * OS + ot * 128:
                                       (p - 1) * OS + (ot + 1) * 128],
                                pows[k][(p, h)][:],
                                start=(k == 0 and p == 1),
                                stop=False,
                            )

        # trailing k-planes group-contiguous: groups finish staggered so
        # bias-add + output DMA overlap the matmul stream
        for ot in range(NT):
            for h in range(NH):
                for k in range(NK - NTAIL, NK):
                    for p in range(1, 5):
                        nc.tensor.matmul(
                            ps[(ot, h)],
                            cts[k][:, (p - 1) * OS + ot * 128:
                                   (p - 1) * OS + (ot + 1) * 128],
                            pows[k][(p, h)][:],
                            start=False,
                            stop=(k == NK - 1 and p == 4),
                        )
                # bias-add split across both engines; output on SP queue
                o_sb = opool.tile([128, 512], F32, tag="o_sb", name=f"o_{ot}_{h}")
                nc.scalar.activation(
                    o_sb[:, 0:256],
                    ps[(ot, h)][:, 0:256],
                    mybir.ActivationFunctionType.Identity,
                    bias=biasc_sb[:, ot:ot + 1],
                )
                nc.vector.tensor_scalar_add(
                    o_sb[:, 256:512], ps[(ot, h)][:, 256:512],
                    biasc_sb[:, ot:ot + 1],
                )
                nc.sync.dma_start(
                    yt[ot * 128:(ot + 1) * 128, h * 512:(h + 1) * 512],
                    o_sb[:],
                )

    nc.compile()
    return nc


def _get_nc():
    if "nc" not in _CACHE:
        _CACHE["nc"] = _build()
    return _CACHE["nc"]


def _make_in_maps(x, coeffs, bias):
    x = np.asarray(x, dtype=np.float32)
    coeffs = np.asarray(coeffs, dtype=np.float32)
    bias = np.asarray(bias, dtype=np.float32)

    # x slices: [1024b, 1024i] -> [1024i, 1024b] -> [8k, 128, 1024]
    xts = [
        np.ascontiguousarray(
            x[bg * BS:(bg + 1) * BS, :].T
        ).reshape(NK, 128, BS)
        for bg in range(BW)
    ]
    # coeff slices: [512o, 1024i, p1..4] -> [1024i, 4p, 512o] bf16
    #   -> [8k, 128, 4*512]
    cts = [
        np.ascontiguousarray(
            coeffs[og * OS:(og + 1) * OS, :, 1:].transpose(1, 2, 0)
        ).astype(ml_dtypes.bfloat16).reshape(NK, 128, 4 * OS)
        for og in range(OW)
    ]
    # biascol[o] = bias[o] + colsum(C0)[o], laid out [128i, 4ot]
    biascs = []
    for og in range(OW):
        bc = (
            bias[0, og * OS:(og + 1) * OS]
            + coeffs[og * OS:(og + 1) * OS, :, 0].sum(axis=1)
        ).astype(np.float32)
        biascs.append(np.ascontiguousarray(bc.reshape(NT, 128).T))
    in_maps = []
    for c in range(BW * OW):
        bg, og = c // OW, c % OW
        in_maps.append({"xt": xts[bg], "ct": cts[og], "biasc": biascs[og]})
    return in_maps


def _gather(results):
    y = np.empty((B, O), dtype=np.float32)
    for c, res in enumerate(results):
        bg, og = c // OW, c % OW
        y[bg * BS:(bg + 1) * BS, og * OS:(og + 1) * OS] = res["yt"].T
    return y


def run(x, coeffs, bias, trace=False, **trace_kwargs):
    nc = _get_nc()
    in_maps = _make_in_maps(x, coeffs, bias)
    br = run_bass_kernel_spmd(
        nc, in_maps, list(range(BW * OW)), trace=trace, **trace_kwargs
    )
    return _gather(br.results), br


def kernel(x, coeffs, bias):
    out, _ = run(x, coeffs, bias)
    return out


# revision 3
# speedup vs baseline: 1.0647x; 1.0647x over previous
"""Trainium2 Bass kernel for KANPolyLayer:
    y[b,o] = sum_{i,p} x[b,i]^p * coeffs[o,i,p] + bias[o],  p = 0..4

Math: y = sum_{p=1..4} (x^p) @ C_p^T + biascol, with C_p = coeffs[:,:,p]
and biascol[o] = bias[o] + sum_i coeffs[o,i,0] folded on host (the p=0
plane is a constant column; folding it is 0.003% of the FLOPs).

All GEMM operands are bf16 (same PE stream rate as fp32r, but FWL
halves LDWEIGHTS and the HBM streams shrink: coeffs 10MB -> 4MB/core,
x 4MB -> 2MB/core).  x is cast to bf16 on host so the DMA'd k-plane IS
the p=1 power; on-chip: p2 = ACT square, p3/p4 = DVE bf16 muls.
Measured end-to-end rel err 7.4e-3 (gate 2e-2).

DMA is split across the two HWDGE queues (SP = x + output, ACT =
coeffs + bias) with ONE descriptor per k-plane (DMA_DIRECT2D costs
~0.6us of issue time each; per-tile descriptors previously burned
~44us of sync-queue time).  First tiles are latency-ordered (k0 split
so the first matmul's operands land first).  A gpsimd-memset-fed PE
warmup starts ~6.3us (right after the framework preamble) so the HAM
clock-gate reaches 2.4 GHz before the real stream begins.

Per-core schedule: 8 PSUM groups (4 o-tiles x 2 b-halves) accumulate
concurrently; each arriving coefficient k-plane unlocks 32 matmuls.
The last NTAIL k-planes are emitted group-contiguous so groups finish
staggered and bias-add + output DMA overlap the stream.  The kernel
computes yT = [o, b]; host transposes.

Sharding (8 cores): 4 batch groups x 2 out-dim groups.
  core c -> (bg, og) = (c // 2, c % 2)
Each core computes a disjoint (512 x 1024) block of yT; host gathers.
"""

from contextlib import ExitStack

import ml_dtypes
import numpy as np

import concourse.bacc as bacc
import concourse.bass as bass
import concourse.mybir as mybir
import concourse.tile as tile
from concourse.bass_utils import run_bass_kernel_spmd

F32 = mybir.dt.float32
BF16 = mybir.dt.bfloat16

B, I, O = 4096, 1024, 1024  # batch, in_dim, out_dim
BW, OW = 4, 2               # batch groups x out-dim groups (8 cores)
BS, OS = B // BW, O // OW   # per-core batch (1024) and out (512)
NK = I // 128               # contraction tiles (8)
NT = OS // 128              # o-tiles (4)
NH = BS // 512              # b-halves (2)
NTAIL = 2                   # trailing k-planes emitted group-contiguous
WN = 8                      # PE warmup matmuls (HAM clock-gate)

_CACHE: dict = {}


def _build():
    nc = bacc.Bacc("TRN2", target_bir_lowering=False, debug=False, num_devices=8)

    # [k, i, b] bf16: per-k contiguous; the DMA'd plane IS the p=1 power
    xt = nc.dram_tensor("xt", [NK, 128, BS], BF16, kind="ExternalInput")
    # [k, i, p*o] bf16: per-k plane holds all 4 power planes' coeff cols
    ct = nc.dram_tensor("ct", [NK, 128, 4 * OS], BF16, kind="ExternalInput")
    # [i, ot]: bias[o] + colsum(C0)[o] as per-partition scalars
    biasc = nc.dram_tensor("biasc", [128, NT], F32, kind="ExternalInput")
    yt = nc.dram_tensor("yt", [OS, BS], F32, kind="ExternalOutput")  # [o, b]

    with tile.TileContext(nc) as tc, ExitStack() as ctx:
        cons = ctx.enter_context(tc.tile_pool(name="cons", bufs=1))
        cpool = ctx.enter_context(tc.tile_pool(name="coef", bufs=3))
        xpool = ctx.enter_context(tc.tile_pool(name="xin", bufs=3))
        ppool = ctx.enter_context(tc.tile_pool(name="pow", bufs=1))
        opool = ctx.enter_context(tc.tile_pool(name="out", bufs=3))
        pspool = ctx.enter_context(
            tc.tile_pool(name="ps", bufs=8, space=bass.MemorySpace.PSUM)
        )

        # 8 concurrent accumulation groups: (o-tile, b-half) -> one PSUM bank
        ps = {}
        for ot in range(NT):
            for h in range(NH):
                ps[(ot, h)] = pspool.tile(
                    [128, 512], F32, tag="ps", name=f"ps_{ot}_{h}"
                )

        # PE warmup: garbage matmuls on a gpsimd-memset tile (gpsimd exits
        # the preamble earliest) so the HAM clock-gate reaches 2.4 GHz
        # before the real stream starts.
        wz = cons.tile([128, 512], BF16)
        nc.gpsimd.memset(wz[:], 0.0)
        for w in range(WN):
            nc.tensor.matmul(
                ps[(0, 0)], wz[:, 0:128], wz[:], start=True, stop=True,
                skip_group_check=True,
            )

        biasc_sb = cons.tile([128, NT], F32)

        pows = {}
        cts = {}
        for k in range(NK):
            # x k-plane (= p1 power) on the SP queue; k0 split in halves so
            # the first matmul's rhs lands first
            xk = xpool.tile([128, BS], BF16, tag="xk", name=f"xk_{k}")
            if k == 0:
                nc.sync.dma_start(xk[:, 0:512], xt[0, :, 0:512])
                nc.sync.dma_start(xk[:, 512:1024], xt[0, :, 512:1024])
            else:
                nc.sync.dma_start(xk[:], xt[k])

            # coeff k-plane on the ACT queue; k0 split so the p=1 slice
            # lands first
            ck = cpool.tile([128, 4 * OS], BF16, tag="ck", name=f"ck_{k}")
            if k == 0:
                nc.scalar.dma_start(ck[:, 0:OS], ct[0, :, 0:OS])
                nc.scalar.dma_start(ck[:, OS:4 * OS], ct[0, :, OS:4 * OS])
                nc.scalar.dma_start(biasc_sb[:], biasc[:])
            else:
                nc.scalar.dma_start(ck[:], ct[k])
            cts[k] = ck

            # powers for this k, per b-half, all bf16 [128, 512]
            pk = {}
            for h in range(NH):
                sl = xk[:, h * 512:(h + 1) * 512]
                p2 = ppool.tile([128, 512], BF16, tag=f"p2_{k}_{h}",
                                name=f"p2_{k}_{h}")
                p3 = ppool.tile([128, 512], BF16, tag=f"p3_{k}_{h}",
                                name=f"p3_{k}_{h}")
                p4 = ppool.tile([128, 512], BF16, tag=f"p4_{k}_{h}",
                                name=f"p4_{k}_{h}")
                nc.scalar.square(p2[:], sl)
                nc.vector.tensor_mul(p3[:], p2[:], sl)
                nc.vector.tensor_mul(p4[:], p2[:], p2[:])
                pk[(1, h)] = sl
                pk[(2, h)] = p2[:]
                pk[(3, h)] = p3[:]
                pk[(4, h)] = p4[:]
            pows[k] = pk

            if k < NK - NTAIL:
                for p in range(1, 5):
                    for ot in range(NT):
                        for h in range(NH):
                            nc.tensor.matmul(
                                ps[(ot, h)],
                                cts[k][:, (p - 1) * OS + ot * 128:
                                       (p - 1) * OS + (ot + 1) * 128],
                                pows[k][(p, h)],
                                start=(k == 0 and p == 1),
                                stop=False,
                            )

        # trailing k-planes group-contiguous: groups finish staggered so
        # bias-add + output DMA overlap the matmul stream
        ngroups = NT * NH
        gi = 0
        for ot in range(NT):
            for h in range(NH):
                for k in range(NK - NTAIL, NK):
                    for p in range(1, 5):
                        nc.tensor.matmul(
                            ps[(ot, h)],
                            cts[k][:, (p - 1) * OS + ot * 128:
                                   (p - 1) * OS + (ot + 1) * 128],
                            pows[k][(p, h)],
                            start=False,
                            stop=(k == NK - 1 and p == 4),
                        )
                # bias-add split across both engines
                o_sb = opool.tile([128, 512], F32, tag="o_sb", name=f"o_{ot}_{h}")
                nc.scalar.activation(
                    o_sb[:, 0:256],
                    ps[(ot, h)][:, 0:256],
                    mybir.ActivationFunctionType.Identity,
                    bias=biasc_sb[:, ot:ot + 1],
                )
                nc.vector.tensor_scalar_add(
                    o_sb[:, 256:512], ps[(ot, h)][:, 256:512],
                    biasc_sb[:, ot:ot + 1],
                )
                gi += 1
                if gi < ngroups:
                    nc.sync.dma_start(
                        yt[ot * 128:(ot + 1) * 128, h * 512:(h + 1) * 512],
                        o_sb[:],
                    )
                else:
                    # last group: split across both queues to halve the tail
                    nc.sync.dma_start(
                        yt[ot * 128:(ot + 1) * 128, h * 512:h * 512 + 256],
                        o_sb[:, 0:256],
                    )
                    nc.scalar.dma_start(
                        yt[ot * 128:(ot + 1) * 128, h * 512 + 256:(h + 1) * 512],
                        o_sb[:, 256:512],
                    )

    nc.compile()
    return nc


def _get_nc():
    if "nc" not in _CACHE:
        _CACHE["nc"] = _build()
    return _CACHE["nc"]


def _make_in_maps(x, coeffs, bias):
    x = np.asarray(x, dtype=np.float32)
    coeffs = np.asarray(coeffs, dtype=np.float32)
    bias = np.asarray(bias, dtype=np.float32)

    # x slices: [1024b, 1024i] -> [1024i, 1024b] bf16 -> [8k, 128, 1024]
    xts = [
        np.ascontiguousarray(x[bg * BS:(bg + 1) * BS, :].T)
        .astype(ml_dtypes.bfloat16)
        .reshape(NK, 128, BS)
        for bg in range(BW)
    ]
    # coeff slices: [512o, 1024i, p1..4] -> [1024i, 4p, 512o] bf16
    #   -> [8k, 128, 4*512]
    cts = [
        np.ascontiguousarray(
            coeffs[og * OS:(og + 1) * OS, :, 1:].transpose(1, 2, 0)
        ).astype(ml_dtypes.bfloat16).reshape(NK, 128, 4 * OS)
        for og in range(OW)
    ]
    # biascol[o] = bias[o] + colsum(C0)[o], laid out [128i, 4ot]
    biascs = []
    for og in range(OW):
        bc = (
            bias[0, og * OS:(og + 1) * OS]
            + coeffs[og * OS:(og + 1) * OS, :, 0].sum(axis=1)
        ).astype(np.float32)
        biascs.append(np.ascontiguousarray(bc.reshape(NT, 128).T))
    in_maps = []
    for c in range(BW * OW):
        bg, og = c // OW, c % OW
        in_maps.append({"xt": xts[bg], "ct": cts[og], "biasc": biascs[og]})
    return in_maps


def _gather(results):
    y = np.empty((B, O), dtype=np.float32)
    for c, res in enumerate(results):
        bg, og = c // OW, c % OW
        y[bg * BS:(bg + 1) * BS, og * OS:(og + 1) * OS] = res["yt"].T
    return y


def run(x, coeffs, bias, trace=False, **trace_kwargs):
    nc = _get_nc()
    in_maps = _make_in_maps(x, coeffs, bias)
    br = run_bass_kernel_spmd(
        nc, in_maps, list(range(BW * OW)), trace=trace, **trace_kwargs
    )
    return _gather(br.results), br


def kernel(x, coeffs, bias):
    out, _ = run(x, coeffs, bias)
    return out
